# revision 11
# baseline (speedup 1.0000x reference)
"""Circulant matmul kernel for Trainium2 (8 NeuronCores, SPMD).

Problem: out = input @ K + bias, where K[c, n] = weight[(c - n) mod 4096],
input is [1024, 4096] f32, weight/bias are [4096] f32.

Strategy (tensor-parallel / column-shard, per the sharding hint):
  - Core c computes out[:, 512c:512(c+1)] = X @ K_c + bias_c in fp32 PSUM.
    No collectives; host concatenates the 8 column slices.
  - All matmul work runs as fp8 DoubleRow pair-matmuls (2 contraction
    chunks per matmul, ~4x the bf16 rate) using a 3-term split:
        x = xh + xl   (xh e4m3, xl = residual in e5m2)
        w = wh + wl   (wh e4m3, wl = residual in e5m2)
        x@K ~= xh@Kh + xh@Kl + xl@Kh      (xl@Kl ~ 2^-8, dropped)
    e5m2's wide exponent range lets the residual terms share the main
    term's PSUM scale, so all three terms accumulate into one PSUM
    group per batch tile.  Measured rel err ~1e-2 vs the 2e-2 gate.
  - Scales keep e4m3 operands out of the subnormal range: x carries
    x*4, K carries w*256, so PSUM holds 1024*out and the epilogue
    multiplies by 2^-10 before adding the unscaled f32 bias.

Device kernel structure (per core):
  - 16 chunk-pairs; per pair: kh/kl on the scalar HWDGE ring, xh/xl on
    the sync ring, in consumption order, so the ~2.1us of matmuls per
    pair overlap the ~2.1us of DMA per pair.
  - PE warm-up: matmuls on a scratch tile whose only writer covers a
    disjoint region, so they carry no dependency, issue the moment the
    Tensor engine enters main, and lift the HAM clock gate while the
    first input chunks are still in flight.
  - Phase 1 processes pairs 0..11 pair-major (matches DMA arrival);
    phase 2 finishes each batch tile in turn over pairs 12..15 so the
    rescale + bias + output-DMA epilogues overlap remaining matmuls.
"""

import numpy as np
import ml_dtypes

import concourse.bass as bass
import concourse.mybir as mybir
import concourse.tile as tile
from concourse import bacc
from concourse.bass import ts
from concourse.bass_utils import run_bass_kernel_spmd

N = 4096
BATCH = 1024
NCORES = 8
NSHARD = N // NCORES          # 512 output columns per core
P = 128                       # partitions
CO = N // P                   # 32 contraction chunks
CP = CO // 2                  # 16 chunk-pairs (DoubleRow does 2 at a time)
BT = BATCH // P               # 8 batch tiles
CP_PH1 = 12                   # pairs processed pair-major in phase 1

SX = 4.0                      # x scale (power of 2)
SW = 256.0                    # w scale (power of 2); SX*SW = 1024
INV_S = 2.0 ** -10

N_WARMUP = 9                  # dummy matmuls to lift the HAM clock gate

BF16 = mybir.dt.bfloat16
FP8E4 = mybir.dt.float8e4
FP8E5 = mybir.dt.float8e5
F32 = mybir.dt.float32
DR = mybir.MatmulPerfMode.DoubleRow


def build_nc():
    """Build the per-core Bass program (same program on all cores; data differs)."""
    nc = bacc.Bacc("TRN2", target_bir_lowering=False, debug=False)

    xh_d = nc.dram_tensor("xh", [N, BATCH], FP8E4, kind="ExternalInput").ap()
    xl_d = nc.dram_tensor("xl", [N, BATCH], FP8E5, kind="ExternalInput").ap()
    kh_d = nc.dram_tensor("kh", [N, NSHARD], FP8E4, kind="ExternalInput").ap()
    kl_d = nc.dram_tensor("kl", [N, NSHARD], FP8E5, kind="ExternalInput").ap()
    bias_d = nc.dram_tensor("biasb", [P, NSHARD], F32, kind="ExternalInput").ap()
    out_d = nc.dram_tensor("out", [BATCH, NSHARD], BF16, kind="ExternalOutput").ap()

    xh_r = xh_d.rearrange("(co ci) b -> ci co b", ci=P)      # [128, 32, 1024]
    xl_r = xl_d.rearrange("(co ci) b -> ci co b", ci=P)
    kh_r = kh_d.rearrange("(co ci) n -> ci co n", ci=P)      # [128, 32, 512]
    kl_r = kl_d.rearrange("(co ci) n -> ci co n", ci=P)

    with tile.TileContext(nc) as tc:
        with (
            tc.tile_pool(name="xhpool", bufs=CP) as xhpool,
            tc.tile_pool(name="xlpool", bufs=CP) as xlpool,
            tc.tile_pool(name="khpool", bufs=CP) as khpool,
            tc.tile_pool(name="klpool", bufs=CP) as klpool,
            tc.tile_pool(name="cpool", bufs=1) as cpool,
            tc.tile_pool(name="tpool", bufs=2) as tpool,
            tc.tile_pool(name="opool", bufs=4) as opool,
            tc.tile_pool(name="psum", bufs=BT, space="PSUM") as psum_pool,
        ):
            # scratch for PE warm-up. Tile requires *a* writer for the tile,
            # but the warm-up matmuls read a region disjoint from the memset
            # so they carry no dependency and start immediately.
            scratch = cpool.tile([P, NSHARD + P], BF16, tag="scratch")
            nc.vector.memset(scratch[:, 0:1], 0.125)

            # per-pair input streams in consumption order
            xh_tiles, xl_tiles, kh_tiles, kl_tiles = [], [], [], []
            bias_sb = None
            for cp in range(CP):
                kht = khpool.tile([P, 2, NSHARD], FP8E4, tag="kh")
                nc.scalar.dma_start(kht[:, 0, :], kh_r[:, 2 * cp, :])
                nc.scalar.dma_start(kht[:, 1, :], kh_r[:, 2 * cp + 1, :])
                kh_tiles.append(kht)
                klt = klpool.tile([P, 2, NSHARD], FP8E5, tag="kl")
                nc.scalar.dma_start(klt[:, 0, :], kl_r[:, 2 * cp, :])
                nc.scalar.dma_start(klt[:, 1, :], kl_r[:, 2 * cp + 1, :])
                kl_tiles.append(klt)
                xht = xhpool.tile([P, 2, BATCH], FP8E4, tag="xh")
                nc.sync.dma_start(xht[:, 0, :], xh_r[:, 2 * cp, :])
                nc.sync.dma_start(xht[:, 1, :], xh_r[:, 2 * cp + 1, :])
                xh_tiles.append(xht)
                xlt = xlpool.tile([P, 2, BATCH], FP8E5, tag="xl")
                nc.sync.dma_start(xlt[:, 0, :], xl_r[:, 2 * cp, :])
                nc.sync.dma_start(xlt[:, 1, :], xl_r[:, 2 * cp + 1, :])
                xl_tiles.append(xlt)
                if cp == CP_PH1:
                    # bias on the scalar ring just before the phase-2 pairs:
                    # in time for the first epilogue, off the critical path
                    bias_sb = cpool.tile([P, NSHARD], F32, tag="bias")
                    nc.scalar.dma_start(bias_sb[:], bias_d)

            psum_tiles = [
                psum_pool.tile([P, NSHARD], F32, tag="ps", name=f"ps{bt}")
                for bt in range(BT)
            ]

            # PE warm-up: full-width dummy matmuls reading garbage
            for i in range(N_WARMUP):
                nc.tensor.matmul(
                    psum_tiles[i % BT][:],
                    scratch[:, P : 2 * P],
                    scratch[:, P : P + NSHARD],
                    start=True,
                    stop=True,
                )

            def pair_mms(cp, bt, start, stop):
                # hh, then hl, then lh — matches per-pair DMA arrival order
                nc.tensor.matmul(
                    psum_tiles[bt][:],
                    xh_tiles[cp][:, :, ts(bt, P)],     # lhsT [c=128, 2, b=128]
                    kh_tiles[cp][:],                   # rhs  [c=128, 2, n=512]
                    start=start,
                    stop=False,
                    perf_mode=DR,
                )
                nc.tensor.matmul(
                    psum_tiles[bt][:],
                    xh_tiles[cp][:, :, ts(bt, P)],
                    kl_tiles[cp][:],
                    start=False,
                    stop=False,
                    perf_mode=DR,
                )
                nc.tensor.matmul(
                    psum_tiles[bt][:],
                    xl_tiles[cp][:, :, ts(bt, P)],
                    kh_tiles[cp][:],
                    start=False,
                    stop=stop,
                    perf_mode=DR,
                )

            # phase 1: pairs 0..CP_PH1-1, pair-major (matches DMA arrival)
            for cp in range(CP_PH1):
                for bt in range(BT):
                    pair_mms(cp, bt, start=(cp == 0), stop=False)

            # phase 2: finish batch tiles one at a time; epilogue overlaps MMs
            for bt in range(BT):
                for cp in range(CP_PH1, CP):
                    pair_mms(cp, bt, start=False, stop=(cp == CP - 1))
                tmp = tpool.tile([P, NSHARD], F32, tag="tmp")
                nc.vector.tensor_scalar_mul(tmp[:], psum_tiles[bt][:], INV_S)
                out_sb = opool.tile([P, NSHARD], BF16, tag="osb")
                nc.vector.tensor_add(out_sb[:], tmp[:], bias_sb[:])
                nc.sync.dma_start(out_d[ts(bt, P), :], out_sb[:])

    nc.compile()
    return nc


def prepare_in_maps(input, weight, bias):
    x = np.asarray(input, dtype=np.float32)
    w = np.asarray(weight, dtype=np.float32)
    b = np.asarray(bias, dtype=np.float32)

    xs = np.ascontiguousarray(x.T) * SX                         # [4096, 1024]
    xh = xs.astype(ml_dtypes.float8_e4m3fn)
    xl = (xs - xh.astype(np.float32)).astype(ml_dtypes.float8_e5m2)

    c = np.arange(N)
    in_maps = []
    for core in range(NCORES):
        n0 = core * NSHARD
        idx = (c[:, None] - (n0 + np.arange(NSHARD))[None, :]) % N
        ks = w[idx] * SW                                        # [4096, 512]
        kh = ks.astype(ml_dtypes.float8_e4m3fn)
        kl = (ks - kh.astype(np.float32)).astype(ml_dtypes.float8_e5m2)
        bias_tile = np.ascontiguousarray(
            np.broadcast_to(b[n0 : n0 + NSHARD].astype(np.float32), (P, NSHARD))
        )
        in_maps.append(
            {"xh": xh, "xl": xl, "kh": kh, "kl": kl, "biasb": bias_tile}
        )
    return in_maps


_NC_CACHE = None


def _get_nc():
    global _NC_CACHE
    if _NC_CACHE is None:
        _NC_CACHE = build_nc()
    return _NC_CACHE


def kernel(**inputs):
    nc = _get_nc()
    in_maps = prepare_in_maps(inputs["input"], inputs["weight"], inputs["bias"])
    res = run_bass_kernel_spmd(nc, in_maps, list(range(NCORES)))
    out = np.empty((BATCH, N), dtype=np.float32)
    for core in range(NCORES):
        out[:, core * NSHARD : (core + 1) * NSHARD] = res.results[core]["out"].astype(
            np.float32
        )
    return out


# revision 12
# speedup vs baseline: 1.3866x; 1.3866x over previous
"""Circulant matmul kernel for Trainium2 (8 NeuronCores, SPMD).

Problem: out = input @ K + bias, where K[c, n] = weight[(c - n) mod 4096],
input is [1024, 4096] f32, weight/bias are [4096] f32.

Strategy (tensor-parallel / column-shard, per the sharding hint):
  - Core c computes out[:, 512c:512(c+1)] = X @ K_c + bias_c in fp32 PSUM.
    No collectives; host concatenates the 8 column slices.
  - Mixed precision on the contraction: 8 of 32 contraction chunks run
    as fp8e4 DoubleRow pair-matmuls.  A matmul instruction costs its
    free-dim cycles (216ns at N=512) regardless of dtype, but a
    DoubleRow pair covers TWO chunks per instruction, so the fp8
    chunks halve their instruction count.  Measured rel err ~1.8e-2
    vs the 2e-2 gate caps the fp8 fraction at 8 chunks.
  - Scales keep every fp8 operand out of e4m3's subnormal range while
    all chunks accumulate into one PSUM group: x carries x*4, K
    carries w*256 (exact powers of two in bf16), so PSUM holds
    1024*out and the epilogue multiplies by 2^-10 before adding the
    unscaled f32 bias.

Device kernel structure (per core):
  - xt chunks on the sync HWDGE ring, kc chunks on the scalar HWDGE
    ring; fp8 pairs interleaved among bf16 chunks in both DMA and
    matmul order so PE demand (a pair is 2x cheaper per DMA'd byte)
    never outruns the DMA rings.
  - PE warm-up: matmuls on a scratch tile whose only writer covers a
    disjoint region, so they carry no dependency, issue the moment the
    Tensor engine enters main, and lift the HAM clock gate while the
    first input chunks are still in flight.
  - Phase 1 runs the interleaved chunk list co-major (matches DMA
    arrival); phase 2 finishes each batch tile in turn (bt-major) so
    the rescale + bias + output-DMA epilogues overlap the remaining
    matmuls.
"""

import numpy as np
import ml_dtypes

import concourse.bass as bass
import concourse.mybir as mybir
import concourse.tile as tile
from concourse import bacc
from concourse.bass import ts
from concourse.bass_utils import run_bass_kernel_spmd

N = 4096
BATCH = 1024
NCORES = 8
NSHARD = N // NCORES          # 512 output columns per core
P = 128                       # partitions
CO = N // P                   # 32 contraction chunks
BT = BATCH // P               # 8 batch tiles

FP8_PAIRS = 4                 # leading chunks done as fp8 DoubleRow pairs
CO8 = 2 * FP8_PAIRS           # fp8 chunks
COB = CO - CO8                # bf16 chunks
COB_PH1 = COB - BT            # bf16 chunks processed co-major in phase 1

SX = 4.0                      # x scale (power of 2)
SW = 256.0                    # w scale (power of 2); SX*SW = 1024
INV_S = 2.0 ** -10

N_WARMUP = 9                  # dummy matmuls to lift the HAM clock gate

BF16 = mybir.dt.bfloat16
FP8 = mybir.dt.float8e4
F32 = mybir.dt.float32


def build_nc():
    """Build the per-core Bass program (same program on all cores; data differs)."""
    nc = bacc.Bacc("TRN2", target_bir_lowering=False, debug=False)

    xt8_d = nc.dram_tensor("xt8", [CO8 * P, BATCH], FP8, kind="ExternalInput").ap()
    kc8_d = nc.dram_tensor("kc8", [CO8 * P, NSHARD], FP8, kind="ExternalInput").ap()
    xt_d = nc.dram_tensor("xt", [COB * P, BATCH], BF16, kind="ExternalInput").ap()
    kc_d = nc.dram_tensor("kc", [COB * P, NSHARD], BF16, kind="ExternalInput").ap()
    bias_d = nc.dram_tensor("biasb", [P, NSHARD], F32, kind="ExternalInput").ap()
    out_d = nc.dram_tensor("out", [BATCH, NSHARD], BF16, kind="ExternalOutput").ap()

    xt8_r = xt8_d.rearrange("(co ci) b -> ci co b", ci=P)    # [128, 8, 1024]
    kc8_r = kc8_d.rearrange("(co ci) n -> ci co n", ci=P)    # [128, 8, 512]
    xt_r = xt_d.rearrange("(co ci) b -> ci co b", ci=P)      # [128, 24, 1024]
    kc_r = kc_d.rearrange("(co ci) n -> ci co n", ci=P)      # [128, 24, 512]

    with tile.TileContext(nc) as tc:
        with (
            tc.tile_pool(name="x8pool", bufs=FP8_PAIRS) as x8pool,
            tc.tile_pool(name="k8pool", bufs=FP8_PAIRS) as k8pool,
            tc.tile_pool(name="xpool", bufs=COB) as xpool,
            tc.tile_pool(name="kpool", bufs=COB) as kpool,
            tc.tile_pool(name="cpool", bufs=1) as cpool,
            tc.tile_pool(name="tpool", bufs=2) as tpool,
            tc.tile_pool(name="opool", bufs=4) as opool,
            tc.tile_pool(name="psum", bufs=BT, space="PSUM") as psum_pool,
        ):
            # scratch for PE warm-up. Tile requires *a* writer for the tile,
            # but the warm-up matmuls read a region disjoint from the memset
            # so they carry no dependency and start immediately.
            scratch = cpool.tile([P, NSHARD + P], BF16, tag="scratch")
            nc.vector.memset(scratch[:, 0:1], 0.125)

            # phase-1 consumption order: fp8 pairs interleaved among bf16
            # chunks so PE demand (a pair is ~2x cheaper per DMA'd byte)
            # never outruns the DMA rings.  DMA issue order matches.
            schedule = []
            pair_after = {0: 0, 1: 2, 2: 4, 3: 6}   # pair p after these b items
            next_p = 0
            for co in range(COB_PH1):
                schedule.append(("b", co))
                while next_p < FP8_PAIRS and pair_after[next_p] == co:
                    schedule.append(("p", next_p))
                    next_p += 1

            x8_tiles = [None] * FP8_PAIRS
            k8_tiles = [None] * FP8_PAIRS
            xt_tiles = [None] * COB
            kc_tiles = [None] * COB
            for kind, i in schedule:
                if kind == "b":
                    ktt = kpool.tile([P, NSHARD], BF16, tag="kc")
                    nc.scalar.dma_start(ktt[:], kc_r[:, i, :])
                    kc_tiles[i] = ktt
                    xtt = xpool.tile([P, BATCH], BF16, tag="xt")
                    nc.sync.dma_start(xtt[:], xt_r[:, i, :])
                    xt_tiles[i] = xtt
                else:
                    k8t = k8pool.tile([P, 2, NSHARD], FP8, tag="kc8")
                    nc.scalar.dma_start(k8t[:, 0, :], kc8_r[:, 2 * i, :])
                    nc.scalar.dma_start(k8t[:, 1, :], kc8_r[:, 2 * i + 1, :])
                    k8_tiles[i] = k8t
                    x8t = x8pool.tile([P, 2, BATCH], FP8, tag="xt8")
                    nc.sync.dma_start(x8t[:, 0, :], xt8_r[:, 2 * i, :])
                    nc.sync.dma_start(x8t[:, 1, :], xt8_r[:, 2 * i + 1, :])
                    x8_tiles[i] = x8t
            # phase-2 bf16 chunks after the interleaved block
            for co in range(COB_PH1, COB):
                ktt = kpool.tile([P, NSHARD], BF16, tag="kc")
                nc.scalar.dma_start(ktt[:], kc_r[:, co, :])
                kc_tiles[co] = ktt
                xtt = xpool.tile([P, BATCH], BF16, tag="xt")
                nc.sync.dma_start(xtt[:], xt_r[:, co, :])
                xt_tiles[co] = xtt
            # bias last on the scalar ring: only needed for the epilogues
            bias_sb = cpool.tile([P, NSHARD], F32, tag="bias")
            nc.scalar.dma_start(bias_sb[:], bias_d)

            psum_tiles = [
                psum_pool.tile([P, NSHARD], F32, tag="ps", name=f"ps{bt}")
                for bt in range(BT)
            ]

            # PE warm-up: full-width dummy matmuls reading garbage
            for i in range(N_WARMUP):
                nc.tensor.matmul(
                    psum_tiles[i % BT][:],
                    scratch[:, P : 2 * P],
                    scratch[:, P : P + NSHARD],
                    start=True,
                    stop=True,
                )

            # phase 1: interleaved fp8 pairs + bf16 chunks, item-major
            for item_idx, (kind, i) in enumerate(schedule):
                for bt in range(BT):
                    if kind == "p":
                        nc.tensor.matmul(
                            psum_tiles[bt][:],
                            x8_tiles[i][:, :, ts(bt, P)],  # lhsT [c=128, 2, b=128]
                            k8_tiles[i][:],                # rhs  [c=128, 2, n=512]
                            start=False,
                            stop=False,
                            perf_mode=mybir.MatmulPerfMode.DoubleRow,
                        )
                    else:
                        nc.tensor.matmul(
                            psum_tiles[bt][:],
                            xt_tiles[i][:, ts(bt, P)],     # lhsT [c=128, b=128]
                            kc_tiles[i][:],                # rhs  [c=128, n=512]
                            start=(item_idx == 0),
                            stop=False,
                        )

            # phase 2: finish batch tiles one at a time; epilogue overlaps MMs
            for bt in range(BT):
                for co in range(COB_PH1, COB):
                    nc.tensor.matmul(
                        psum_tiles[bt][:],
                        xt_tiles[co][:, ts(bt, P)],
                        kc_tiles[co][:],
                        start=False,
                        stop=(co == COB - 1),
                    )
                tmp = tpool.tile([P, NSHARD], F32, tag="tmp")
                nc.vector.tensor_scalar_mul(tmp[:], psum_tiles[bt][:], INV_S)
                out_sb = opool.tile([P, NSHARD], BF16, tag="osb")
                nc.vector.tensor_add(out_sb[:], tmp[:], bias_sb[:])
                nc.sync.dma_start(out_d[ts(bt, P), :], out_sb[:])

    nc.compile()
    return nc


def prepare_in_maps(input, weight, bias):
    x = np.asarray(input, dtype=np.float32)
    w = np.asarray(weight, dtype=np.float32)
    b = np.asarray(bias, dtype=np.float32)

    xs = np.ascontiguousarray(x.T) * SX                         # [4096, 1024]
    xt8 = xs[: CO8 * P].astype(ml_dtypes.float8_e4m3fn)
    xtb = xs[CO8 * P :].astype(ml_dtypes.bfloat16)

    c = np.arange(N)
    in_maps = []
    for core in range(NCORES):
        n0 = core * NSHARD
        idx = (c[:, None] - (n0 + np.arange(NSHARD))[None, :]) % N
        ks = w[idx] * SW                                        # [4096, 512]
        kc8 = ks[: CO8 * P].astype(ml_dtypes.float8_e4m3fn)
        kcb = ks[CO8 * P :].astype(ml_dtypes.bfloat16)
        bias_tile = np.ascontiguousarray(
            np.broadcast_to(b[n0 : n0 + NSHARD].astype(np.float32), (P, NSHARD))
        )
        in_maps.append(
            {"xt8": xt8, "kc8": kc8, "xt": xtb, "kc": kcb, "biasb": bias_tile}
        )
    return in_maps


_NC_CACHE = None


def _get_nc():
    global _NC_CACHE
    if _NC_CACHE is None:
        _NC_CACHE = build_nc()
    return _NC_CACHE


def kernel(**inputs):
    nc = _get_nc()
    in_maps = prepare_in_maps(inputs["input"], inputs["weight"], inputs["bias"])
    res = run_bass_kernel_spmd(nc, in_maps, list(range(NCORES)))
    out = np.empty((BATCH, N), dtype=np.float32)
    for core in range(NCORES):
        out[:, core * NSHARD : (core + 1) * NSHARD] = res.results[core]["out"].astype(
            np.float32
        )
    return out


# revision 15
# speedup vs baseline: 1.4520x; 1.0472x over previous
"""Circulant matmul kernel for Trainium2 (8 NeuronCores, SPMD).

Problem: out = input @ K + bias, where K[c, n] = weight[(c - n) mod 4096],
input is [1024, 4096] f32, weight/bias are [4096] f32.

Strategy (tensor-parallel / column-shard, per the sharding hint):
  - Core c computes out[:, 512c:512(c+1)] = X @ K_c + bias_c in fp32 PSUM.
    No collectives; host concatenates the 8 column slices.
  - Mixed precision on the contraction: 8 of 32 contraction chunks run
    as fp8e4 DoubleRow pair-matmuls.  A matmul instruction costs its
    free-dim cycles (216ns at N=512) regardless of dtype, but a
    DoubleRow pair covers TWO chunks per instruction, so the fp8
    chunks halve their instruction count.  Measured rel err ~1.8e-2
    vs the 2e-2 gate caps the fp8 fraction at 8 chunks.
  - Scales keep every fp8 operand out of e4m3's subnormal range while
    all chunks accumulate into one PSUM group: x carries x*4, K
    carries w*256 (exact powers of two in bf16), so PSUM holds
    1024*out and the epilogue multiplies by 2^-10 before adding the
    unscaled f32 bias.

Device kernel structure (per core):
  - xt chunks on the sync HWDGE ring, kc chunks on the scalar HWDGE
    ring; fp8 pairs interleaved among bf16 chunks in both DMA and
    matmul order so PE demand (a pair is 2x cheaper per DMA'd byte)
    never outruns the DMA rings.
  - PE warm-up: matmuls on a scratch tile whose only writer covers a
    disjoint region, so they carry no dependency, issue the moment the
    Tensor engine enters main, and lift the HAM clock gate while the
    first input chunks are still in flight.
  - Phase 1 runs the interleaved chunk list co-major (matches DMA
    arrival); phase 2 finishes each batch tile in turn (bt-major) so
    the rescale + bias + output-DMA epilogues overlap the remaining
    matmuls.
"""

import numpy as np
import ml_dtypes

import concourse.bass as bass
import concourse.mybir as mybir
import concourse.tile as tile
from concourse import bacc
from concourse.bass import ts
from concourse.bass_utils import run_bass_kernel_spmd

N = 4096
BATCH = 1024
NCORES = 8
NSHARD = N // NCORES          # 512 output columns per core
P = 128                       # partitions
CO = N // P                   # 32 contraction chunks
BT = BATCH // P               # 8 batch tiles

FP8_PAIRS = 4                 # leading chunks done as fp8 DoubleRow pairs
CO8 = 2 * FP8_PAIRS           # fp8 chunks
COB = CO - CO8                # bf16 chunks
COB_PH1 = COB - BT            # bf16 chunks processed co-major in phase 1

SX = 4.0                      # x scale (power of 2)
SW = 256.0                    # w scale (power of 2); SX*SW = 1024
INV_S = 2.0 ** -10

N_WARMUP = 9                  # dummy matmuls to lift the HAM clock gate

BF16 = mybir.dt.bfloat16
FP8 = mybir.dt.float8e4
F32 = mybir.dt.float32


def build_nc():
    """Build the per-core Bass program (same program on all cores; data differs)."""
    nc = bacc.Bacc("TRN2", target_bir_lowering=False, debug=False)

    xt8_d = nc.dram_tensor("xt8", [CO8 * P, BATCH], FP8, kind="ExternalInput").ap()
    kc8_d = nc.dram_tensor("kc8", [CO8 * P, NSHARD], FP8, kind="ExternalInput").ap()
    xt_d = nc.dram_tensor("xt", [COB * P, BATCH], BF16, kind="ExternalInput").ap()
    kc_d = nc.dram_tensor("kc", [COB * P, NSHARD], BF16, kind="ExternalInput").ap()
    bias_d = nc.dram_tensor("biasb", [P, NSHARD], F32, kind="ExternalInput").ap()
    out_d = nc.dram_tensor("out", [BATCH, NSHARD], BF16, kind="ExternalOutput").ap()

    xt8_r = xt8_d.rearrange("(co ci) b -> ci co b", ci=P)    # [128, 8, 1024]
    kc8_r = kc8_d.rearrange("(co ci) n -> ci co n", ci=P)    # [128, 8, 512]
    xt_r = xt_d.rearrange("(co ci) b -> ci co b", ci=P)      # [128, 24, 1024]
    kc_r = kc_d.rearrange("(co ci) n -> ci co n", ci=P)      # [128, 24, 512]

    with tile.TileContext(nc) as tc:
        with (
            tc.tile_pool(name="x8pool", bufs=FP8_PAIRS) as x8pool,
            tc.tile_pool(name="k8pool", bufs=FP8_PAIRS) as k8pool,
            tc.tile_pool(name="xpool", bufs=COB) as xpool,
            tc.tile_pool(name="kpool", bufs=COB) as kpool,
            tc.tile_pool(name="cpool", bufs=1) as cpool,
            tc.tile_pool(name="tpool", bufs=2) as tpool,
            tc.tile_pool(name="opool", bufs=4) as opool,
            tc.tile_pool(name="psum", bufs=BT, space="PSUM") as psum_pool,
        ):
            # scratch for PE warm-up. Tile requires *a* writer for the tile,
            # but the warm-up matmuls read a region disjoint from the memset
            # so they carry no dependency and start immediately.
            scratch = cpool.tile([P, NSHARD + P], BF16, tag="scratch")
            nc.vector.memset(scratch[:, 0:1], 0.125)

            # phase-1 consumption order: fp8 pairs interleaved among bf16
            # chunks so PE demand (a pair is ~2x cheaper per DMA'd byte)
            # never outruns the DMA rings.  DMA issue order matches.
            schedule = []
            pair_after = {0: 1, 1: 3, 2: 5, 3: 7}   # pair p after these b items
            next_p = 0
            for co in range(COB_PH1):
                schedule.append(("b", co))
                while next_p < FP8_PAIRS and pair_after[next_p] == co:
                    schedule.append(("p", next_p))
                    next_p += 1

            x8_tiles = [None] * FP8_PAIRS
            k8_tiles = [None] * FP8_PAIRS
            xt_tiles = [None] * COB
            kc_tiles = [None] * COB
            for kind, i in schedule:
                if kind == "b":
                    ktt = kpool.tile([P, NSHARD], BF16, tag="kc")
                    nc.scalar.dma_start(ktt[:], kc_r[:, i, :])
                    kc_tiles[i] = ktt
                    xtt = xpool.tile([P, BATCH], BF16, tag="xt")
                    nc.sync.dma_start(xtt[:], xt_r[:, i, :])
                    xt_tiles[i] = xtt
                else:
                    k8t = k8pool.tile([P, 2, NSHARD], FP8, tag="kc8")
                    nc.scalar.dma_start(k8t[:, 0, :], kc8_r[:, 2 * i, :])
                    nc.scalar.dma_start(k8t[:, 1, :], kc8_r[:, 2 * i + 1, :])
                    k8_tiles[i] = k8t
                    x8t = x8pool.tile([P, 2, BATCH], FP8, tag="xt8")
                    nc.sync.dma_start(x8t[:, 0, :], xt8_r[:, 2 * i, :])
                    nc.sync.dma_start(x8t[:, 1, :], xt8_r[:, 2 * i + 1, :])
                    x8_tiles[i] = x8t
            # phase-2 bf16 chunks after the interleaved block
            for co in range(COB_PH1, COB):
                ktt = kpool.tile([P, NSHARD], BF16, tag="kc")
                nc.scalar.dma_start(ktt[:], kc_r[:, co, :])
                kc_tiles[co] = ktt
                xtt = xpool.tile([P, BATCH], BF16, tag="xt")
                nc.sync.dma_start(xtt[:], xt_r[:, co, :])
                xt_tiles[co] = xtt
            # bias last on the scalar ring: only needed for the epilogues
            bias_sb = cpool.tile([P, NSHARD], F32, tag="bias")
            nc.scalar.dma_start(bias_sb[:], bias_d)

            psum_tiles = [
                psum_pool.tile([P, NSHARD], F32, tag="ps", name=f"ps{bt}")
                for bt in range(BT)
            ]

            # PE warm-up: full-width dummy matmuls reading garbage
            for i in range(N_WARMUP):
                nc.tensor.matmul(
                    psum_tiles[i % BT][:],
                    scratch[:, P : 2 * P],
                    scratch[:, P : P + NSHARD],
                    start=True,
                    stop=True,
                )

            def bf_mm(co, bt, start=False, stop=False):
                nc.tensor.matmul(
                    psum_tiles[bt][:],
                    xt_tiles[co][:, ts(bt, P)],        # lhsT [c=128, b=128]
                    kc_tiles[co][:],                   # rhs  [c=128, n=512]
                    start=start,
                    stop=stop,
                )

            def dr_mm(p, bt):
                nc.tensor.matmul(
                    psum_tiles[bt][:],
                    x8_tiles[p][:, :, ts(bt, P)],      # lhsT [c=128, 2, b=128]
                    k8_tiles[p][:],                    # rhs  [c=128, 2, n=512]
                    start=False,
                    stop=False,
                    perf_mode=mybir.MatmulPerfMode.DoubleRow,
                )

            # phase 1: DoubleRow pair MMs are zipped 1:1 with an adjacent
            # bf16 chunk's MMs — a bf16 MM between any two DoubleRow MMs
            # keeps the HAM activity monitor fed (DoubleRow bursts read as
            # idle and trigger mid-stream re-throttle oscillation).
            zip_with = {1: 0, 3: 1, 5: 2, 7: 3}        # bf16 co -> pair idx
            for co in range(COB_PH1):
                if co in zip_with:
                    p = zip_with[co]
                    for bt in range(BT):
                        bf_mm(co, bt)
                        dr_mm(p, bt)
                else:
                    for bt in range(BT):
                        bf_mm(co, bt, start=(co == 0))

            # phase 2: finish batch tiles one at a time; epilogue overlaps MMs
            for bt in range(BT):
                for co in range(COB_PH1, COB):
                    nc.tensor.matmul(
                        psum_tiles[bt][:],
                        xt_tiles[co][:, ts(bt, P)],
                        kc_tiles[co][:],
                        start=False,
                        stop=(co == COB - 1),
                    )
                tmp = tpool.tile([P, NSHARD], F32, tag="tmp")
                nc.vector.tensor_scalar_mul(tmp[:], psum_tiles[bt][:], INV_S)
                out_sb = opool.tile([P, NSHARD], BF16, tag="osb")
                nc.vector.tensor_add(out_sb[:], tmp[:], bias_sb[:])
                nc.sync.dma_start(out_d[ts(bt, P), :], out_sb[:])

    nc.compile()
    return nc


def prepare_in_maps(input, weight, bias):
    x = np.asarray(input, dtype=np.float32)
    w = np.asarray(weight, dtype=np.float32)
    b = np.asarray(bias, dtype=np.float32)

    xs = np.ascontiguousarray(x.T) * SX                         # [4096, 1024]
    xt8 = xs[: CO8 * P].astype(ml_dtypes.float8_e4m3fn)
    xtb = xs[CO8 * P :].astype(ml_dtypes.bfloat16)

    c = np.arange(N)
    in_maps = []
    for core in range(NCORES):
        n0 = core * NSHARD
        idx = (c[:, None] - (n0 + np.arange(NSHARD))[None, :]) % N
        ks = w[idx] * SW                                        # [4096, 512]
        kc8 = ks[: CO8 * P].astype(ml_dtypes.float8_e4m3fn)
        kcb = ks[CO8 * P :].astype(ml_dtypes.bfloat16)
        bias_tile = np.ascontiguousarray(
            np.broadcast_to(b[n0 : n0 + NSHARD].astype(np.float32), (P, NSHARD))
        )
        in_maps.append(
            {"xt8": xt8, "kc8": kc8, "xt": xtb, "kc": kcb, "biasb": bias_tile}
        )
    return in_maps


_NC_CACHE = None


def _get_nc():
    global _NC_CACHE
    if _NC_CACHE is None:
        _NC_CACHE = build_nc()
    return _NC_CACHE


def kernel(**inputs):
    nc = _get_nc()
    in_maps = prepare_in_maps(inputs["input"], inputs["weight"], inputs["bias"])
    res = run_bass_kernel_spmd(nc, in_maps, list(range(NCORES)))
    out = np.empty((BATCH, N), dtype=np.float32)
    for core in range(NCORES):
        out[:, core * NSHARD : (core + 1) * NSHARD] = res.results[core]["out"].astype(
            np.float32
        )
    return out
